# revision 31
# baseline (speedup 1.0000x reference)
"""BiLSTM-CRF negative log-likelihood kernel for 8 Trainium2 NeuronCores.

Strategy (data parallel over batch, 64 sequences per core):

logZ via PARALLEL SEGMENTS: the CRF forward chain contracts in direction
~10x per step (Birkhoff), so each sequence's 2048-step chain is split into
G=51 segments (seg 0: steps [0,48) started exactly from e_START; segs g>=1:
window [40g+8, 40g+48) preceded by K=8 burn-in steps from a uniform vector).
All 51*64 = 3264 segment-chains per core run simultaneously as columns of
[128, 816] tiles (4 groups of 32 tags stacked on partitions), so the serial
depth is 48 matmul+mul steps instead of 2048. Per-column log-growth between
two "captures" (V^T @ state matmuls at idx 8 and 48, with a plain-sum row
and an exp(trans[STOP]) -weighted row per group) telescopes into logZ.
No renorm is needed over 48 steps (bf16/fp32 exponent range suffices);
host adds back the constant MU per step and sums.

Gold score: emission = sum(feats * onehot(tags)) on device (multiply on
gpsimd, per-partition reduce on the scalar engine via activation accum;
last chunk as fused vector stt pieces in the chain-mul wait gaps);
transitions = <trans, C> on device where C is the [32,32] tag-pair count
histogram (integer preprocessing of tags, marshalled host-side like the
one-hot masks); stop term via a masked reduce of trans[STOP,:]. Device
outputs ln-captures and gold partials; host sums.

All inputs ride 3 DMAs: one combined per-chunk stream (raw feats slots +
window-only one-hot slots interleaved per chunk) and two packed param
tensors - dma_start issue time on the Sync queue is ~600 ns each, so
fewer, bigger DMAs matter.
"""

import sys

sys.path.insert(0, "/opt/trn_rl_repo")

import numpy as np
import ml_dtypes

B, S, T = 512, 2048, 32
START_IDX, STOP_IDX = 30, 31
N_CORES = 8
BC = B // N_CORES           # 64 sequences per core
K_BURN = 8
L_WIN = 40
DEPTH = K_BURN + L_WIN      # 48 serial steps
G_SEG = 51                  # 2048 = DEPTH + (G_SEG-1)*L_WIN
C_TOT = G_SEG * BC          # 3264 columns
NGRP = 4                    # tag-groups stacked on partitions
CPG = C_TOT // NGRP         # 816 columns per partition-group row
NSTR = 2                    # independent chain streams
CPS = CPG // NSTR           # 408 columns per stream
IC = 8                      # idx per streamed chunk
NCH = DEPTH // IC           # 6 chunks
HC = IC // 2
MU = float(np.log(32.0) + 1.0)
NEG = -10000.0
CCORR = BC * K_BURN // NGRP  # 128 corr-cell columns
# combined stream slot layout: chunk 0 -> 8 raw slots; chunks 1..5 ->
# 8 raw slots + 8 mc slots each
COMB_SLOTS = IC + (NCH - 1) * 2 * IC   # 88
# packed fp32 params: transT4 | tstop | cmat | trans
P32_W = 128 + 1 + T + T
# packed bf16 params: init | maskstop | rawc | mcc
PBF_W = CPG + BC + CCORR + CCORR

BF16 = ml_dtypes.bfloat16


def _build_program():
    import concourse.bass as bass
    import concourse.tile as tile
    from concourse import bacc, mybir

    dt = mybir.dt
    AF = mybir.ActivationFunctionType
    ALU = mybir.AluOpType
    AX = mybir.AxisListType

    nc = bacc.Bacc("TRN2", target_bir_lowering=False, debug=False,
                   num_devices=N_CORES)

    rawmc_d = nc.dram_tensor("rawmc", [128, COMB_SLOTS, CPG], dt.bfloat16,
                             kind="ExternalInput").ap()
    p32_d = nc.dram_tensor("p32", [128, P32_W], dt.float32,
                           kind="ExternalInput").ap()
    pbf_d = nc.dram_tensor("pbf", [128, PBF_W], dt.bfloat16,
                           kind="ExternalInput").ap()

    caps_d = nc.dram_tensor("caps", [8, 2 * CPG], dt.float32,
                            kind="ExternalOutput").ap()
    goldp_d = nc.dram_tensor("goldp", [128, 1], dt.float32,
                             kind="ExternalOutput").ap()

    with tile.TileContext(nc) as tc:
        with (
            tc.tile_pool(name="singles", bufs=1) as singles,
            tc.tile_pool(name="stateA", bufs=2) as stateA,
            tc.tile_pool(name="stateB", bufs=2) as stateB,
            tc.tile_pool(name="combp", bufs=3) as combp,
            tc.tile_pool(name="ftp", bufs=3) as ftp,
            tc.tile_pool(name="scrp", bufs=2) as scrp,
            tc.tile_pool(name="scrp2", bufs=1) as scrp2,
            tc.tile_pool(name="tailp", bufs=1) as tailp,
            tc.tile_pool(name="psA", bufs=1, space="PSUM") as psA,
            tc.tile_pool(name="psB", bufs=1, space="PSUM") as psB,
            tc.tile_pool(name="psc", bufs=2, space="PSUM") as psc,
        ):
            # ---------- chunk-0 stream DMA first, then params ----------
            comb0 = combp.tile([128, IC, CPG], dt.bfloat16, tag="c0")
            nc.sync.dma_start(comb0[:, :, :], rawmc_d[:, 0:IC, :])
            pbf = singles.tile([128, PBF_W], dt.bfloat16)
            nc.sync.dma_start(pbf[:, :], pbf_d[:, :])
            p32 = singles.tile([128, P32_W], dt.float32)
            nc.sync.dma_start(p32[:, :], p32_d[:, :])

            maskstop = pbf[0:T, CPG:CPG + BC]
            rawc = pbf[:, CPG + BC:CPG + BC + CCORR]
            mcc = pbf[:, CPG + BC + CCORR:PBF_W]
            tstop = p32[0:T, 128:129]
            cmat = p32[0:T, 129:129 + T]
            trans32 = p32[0:T, 129 + T:P32_W]

            # chain stationary: blockdiag4(exp(transT)) in bf16
            E4 = singles.tile([128, 128], dt.bfloat16)
            nc.scalar.activation(E4[:, :], p32[:, 0:128], AF.Exp)

            mub = singles.tile([128, 1], dt.float32)
            nc.vector.memset(mub[:, :], -MU)

            # capture stationary V [128, 8]: col 2q = ones on group q,
            # col 2q+1 = exp(trans[STOP,:]) on group q
            V = singles.tile([128, 8], dt.bfloat16)
            nc.vector.memset(V[:, :], 0.0)
            for q in range(NGRP):
                nc.vector.memset(V[32 * q:32 * q + 32, 2 * q:2 * q + 1], 1.0)
                nc.scalar.activation(V[32 * q:32 * q + 32, 2 * q + 1:2 * q + 2],
                                     tstop, AF.Exp)

            # gold accumulator slots
            NSLOT = 24
            eaccs = singles.tile([128, NSLOT], dt.float32)
            nc.vector.memset(eaccs[:, :], 0.0)
            slot = [0]

            def next_slot():
                s = slot[0]
                slot[0] += 1
                assert s < NSLOT
                return s

            # initial states
            stA = stateA.tile([128, CPS], dt.bfloat16, tag="stA")
            nc.vector.tensor_copy(stA[:, :], pbf[:, 0:CPS])
            stB = stateB.tile([128, CPS], dt.bfloat16, tag="stB")
            nc.vector.tensor_copy(stB[:, :], pbf[:, CPS:CPG])

            # captures land in SBUF immediately (PSUM bank budget)
            capsb = singles.tile([8, 2 * CPG], dt.float32)

            def do_capture(cidx, sA, sB):
                for s, st_s in ((0, sA), (1, sB)):
                    pc = psc.tile([8, CPS], dt.float32, tag="cap")
                    nc.tensor.matmul(pc[:, :], V[:, :], st_s[:, :],
                                     start=True, stop=True)
                    dst = capsb[:, cidx * CPG + s * CPS:
                                cidx * CPG + (s + 1) * CPS]
                    nc.vector.tensor_copy(dst, pc[:, :])

            pending_reduce = []

            # ---------- main loop ----------
            for ck in range(NCH):
                i0 = ck * IC
                if ck == 0:
                    comb = comb0
                else:
                    comb = combp.tile([128, 2 * IC, CPG], dt.bfloat16,
                                      tag="comb")
                    s0 = IC + (ck - 1) * 2 * IC
                    nc.sync.dma_start(comb[:, :, :],
                                      rawmc_d[:, s0:s0 + 2 * IC, :])
                raw = comb[:, 0:IC, :]
                mc = None if ck == 0 else comb[:, IC:2 * IC, :]

                # emission multiplies on gpsimd; reduces go to the scalar
                # engine one chunk later. Last chunk: fused vector stt
                # pieces interleaved into the chain-mul wait gaps.
                if 1 <= ck < NCH - 1:
                    scr = scrp.tile([128, IC, CPG], dt.bfloat16, tag="scr")
                    nc.gpsimd.tensor_mul(scr[:, :, :],
                                         comb[:, IC:2 * IC, :],
                                         comb[:, 0:IC, :])
                    pending_reduce.append(scr[:, 0:HC, :])
                    pending_reduce.append(scr[:, HC:IC, :])

                # chain: 8 steps, 2 streams; ftile in half-chunk slices
                for j in range(IC):
                    i = i0 + j
                    if j % HC == 0:
                        ftile = ftp.tile([128, HC, CPG], dt.bfloat16,
                                         tag="ft")
                        nc.scalar.activation(ftile[:, :, :],
                                             comb[:, j:j + HC, :], AF.Exp,
                                             bias=mub[:, :])
                    if i == K_BURN:
                        do_capture(0, stA, stB)
                    puA = psA.tile([128, CPS], dt.float32, tag="puA")
                    nc.tensor.matmul(puA[:, :], E4[:, :], stA[:, :],
                                     start=True, stop=True)
                    puB = psB.tile([128, CPS], dt.float32, tag="puB")
                    nc.tensor.matmul(puB[:, :], E4[:, :], stB[:, :],
                                     start=True, stop=True)
                    stA = stateA.tile([128, CPS], dt.bfloat16, tag="stA")
                    nc.vector.tensor_mul(stA[:, :], puA[:, :],
                                         ftile[:, j % HC, 0:CPS])
                    stB = stateB.tile([128, CPS], dt.bfloat16, tag="stB")
                    nc.vector.tensor_mul(stB[:, :], puB[:, :],
                                         ftile[:, j % HC, CPS:CPG])
                    if ck == NCH - 1:
                        scr = scrp2.tile([128, CPG], dt.bfloat16, tag="scrl")
                        nc.vector.scalar_tensor_tensor(
                            scr[:, :], raw[:, j, :], 1.0, mc[:, j, :],
                            op0=ALU.mult, op1=ALU.mult,
                            accum_out=eaccs[:, next_slot():slot[0]])
                    if j == HC and pending_reduce:
                        for scr_ap in pending_reduce:
                            scr2 = scrp2.tile([128, HC, CPG], dt.bfloat16,
                                              tag="sc2")
                            nc.scalar.activation(
                                scr2[:, :, :], scr_ap, AF.Identity,
                                accum_out=eaccs[:, next_slot():slot[0]])
                        pending_reduce.clear()

            for scr_ap in pending_reduce:
                scr2 = scrp2.tile([128, HC, CPG], dt.bfloat16, tag="sc2")
                nc.scalar.activation(scr2[:, :, :], scr_ap, AF.Identity,
                                     accum_out=eaccs[:, next_slot():slot[0]])
            pending_reduce.clear()

            # capture 2: final states
            do_capture(1, stA, stB)

            # ---------- gold tails ----------
            scr2 = tailp.tile([128, CCORR], dt.bfloat16)
            nc.vector.scalar_tensor_tensor(
                scr2[:, :], rawc, 1.0, mcc,
                op0=ALU.mult, op1=ALU.mult,
                accum_out=eaccs[:, next_slot():slot[0]])
            scr3 = tailp.tile([T, T], dt.float32)
            nc.vector.scalar_tensor_tensor(
                scr3[:, :], trans32, 1.0, cmat,
                op0=ALU.mult, op1=ALU.mult,
                accum_out=eaccs[0:T, next_slot():slot[0]])
            scr4 = tailp.tile([T, BC], dt.float32)
            nc.vector.scalar_tensor_tensor(
                scr4[:, :], maskstop, tstop, maskstop,
                op0=ALU.mult, op1=ALU.mult,
                accum_out=eaccs[0:T, next_slot():slot[0]])

            gp = tailp.tile([128, 1], dt.float32)
            nc.vector.tensor_reduce(gp[:, :], eaccs[:, :], axis=AX.X,
                                    op=ALU.add)
            nc.sync.dma_start(goldp_d[:, :], gp[:, :])

            # ---------- captures: ln + out ----------
            capln = tailp.tile([8, 2 * CPG], dt.float32)
            nc.scalar.activation(capln[:, :], capsb[:, :], AF.Ln)
            nc.sync.dma_start(caps_d[:, :], capln[:, :])

    nc.compile()
    return nc


def _marshal(feats, transitions, tags):
    feats = np.asarray(feats, dtype=np.float32)
    trans = np.asarray(transitions, dtype=np.float32)
    tags = np.asarray(tags)
    eye = np.arange(T, dtype=tags.dtype)

    g_of_c = np.arange(C_TOT) // BC
    b_of_c = np.arange(C_TOT) % BC
    # cell time: t(i, g) = L_WIN*g + i
    t_cell = (L_WIN * g_of_c)[:, None] + np.arange(DEPTH)[None, :]  # [C,48]

    p32 = np.full((128, P32_W), NEG, dtype=np.float32)
    tq = np.ascontiguousarray(trans.T)
    for q in range(NGRP):
        p32[32 * q:32 * q + 32, 32 * q:32 * q + 32] = tq
    p32[:, 128:] = 0.0
    p32[0:T, 128] = trans[STOP_IDX, :]
    p32[0:T, 129 + T:P32_W] = trans

    in_maps = []
    for c in range(N_CORES):
        b0, b1 = c * BC, (c + 1) * BC
        f = feats[b0:b1]          # [64, 2048, 32]
        tg = tags[b0:b1]          # [64, 2048]

        cells = f[b_of_c[:, None], t_cell, :]            # [C, 48, 32]
        raw = cells.reshape(NGRP, CPG, DEPTH, T) \
            .transpose(0, 3, 2, 1).reshape(128, DEPTH, CPG).astype(BF16)

        tw = tg[b_of_c[:, None], t_cell[:, K_BURN:]]     # [C, 40] window
        mcw = (tw[:, :, None] == eye).astype(BF16)        # [C, 40, 32]
        mcw = mcw.reshape(NGRP, CPG, DEPTH - K_BURN, T) \
            .transpose(0, 3, 2, 1).reshape(128, DEPTH - K_BURN, CPG)

        rawmc = np.empty((128, COMB_SLOTS, CPG), dtype=BF16)
        rawmc[:, 0:IC] = raw[:, 0:IC]
        for ck in range(1, NCH):
            s0 = IC + (ck - 1) * 2 * IC
            rawmc[:, s0:s0 + IC] = raw[:, ck * IC:(ck + 1) * IC]
            rawmc[:, s0 + IC:s0 + 2 * IC] = \
                mcw[:, (ck - 1) * IC:ck * IC]

        init_cols = np.full((C_TOT, T), 1.0 / T, dtype=np.float32)
        init_cols[g_of_c == 0] = 0.0
        init_cols[g_of_c == 0, START_IDX] = 1.0
        init = init_cols.reshape(NGRP, CPG, T).transpose(0, 2, 1) \
            .reshape(128, CPG)

        bb = np.arange(BC * K_BURN) // K_BURN
        tt = np.arange(BC * K_BURN) % K_BURN
        rawc = f[bb, tt, :].reshape(NGRP, CCORR, T).transpose(0, 2, 1) \
            .reshape(128, CCORR)
        mcc = (tg[bb, tt][:, None] == eye) \
            .reshape(NGRP, CCORR, T).transpose(0, 2, 1).reshape(128, CCORR)

        pbf = np.zeros((128, PBF_W), dtype=BF16)
        pbf[:, 0:CPG] = init.astype(BF16)
        pbf[0:T, CPG:CPG + BC] = (tg[:, S - 1, None] == eye).T.astype(BF16)
        pbf[:, CPG + BC:CPG + BC + CCORR] = rawc.astype(BF16)
        pbf[:, CPG + BC + CCORR:PBF_W] = mcc.astype(BF16)

        # pair-count histogram C[i, j] = #{t: tag_t = i, tag_{t-1} = j}
        prev = np.concatenate(
            [np.full((BC, 1), START_IDX, dtype=tg.dtype), tg[:, :-1]], axis=1)
        pair = (tg.astype(np.int64) * T + prev.astype(np.int64)).ravel()
        cmat = np.bincount(pair, minlength=T * T).reshape(T, T)
        p32c = p32.copy()
        p32c[0:T, 129:129 + T] = cmat.astype(np.float32)

        in_maps.append({"rawmc": rawmc, "p32": p32c, "pbf": pbf})
    return in_maps


_PROGRAM = [None]
TRACE = False
TRACE_KW = {}
LAST_EXEC_NS = None
LAST_RESULT = [None]

_G_OF_C = np.arange(C_TOT) // BC
_GRP_OF_C = np.arange(C_TOT) // CPG
_J_OF_C = np.arange(C_TOT) % CPG
_S_OF_C = _J_OF_C // CPS
_JJ_OF_C = _J_OF_C % CPS


def kernel(feats, transitions, tags):
    global LAST_EXEC_NS
    from concourse.bass_utils import run_bass_kernel_spmd

    if _PROGRAM[0] is None:
        _PROGRAM[0] = _build_program()
    nc = _PROGRAM[0]
    in_maps = _marshal(feats, transitions, tags)
    res = run_bass_kernel_spmd(nc, in_maps, list(range(N_CORES)),
                               trace=TRACE, **TRACE_KW)
    LAST_EXEC_NS = res.exec_time_ns
    LAST_RESULT[0] = res

    col_idx = _GRP_OF_C * 2
    col_idx_rw = _GRP_OF_C * 2 + 1
    cap1_col = 0 * CPG + _S_OF_C * CPS + _JJ_OF_C
    cap2_col = 1 * CPG + _S_OF_C * CPS + _JJ_OF_C
    is_last = _G_OF_C == G_SEG - 1
    mu_corr = np.where(_G_OF_C == 0, DEPTH * MU, L_WIN * MU)

    total = 0.0
    for c in range(N_CORES):
        r = res.results[c]
        caps = r["caps"]            # [8, 2*CPG]
        ln1 = caps[col_idx, cap1_col]
        ln2p = caps[col_idx, cap2_col]
        ln2r = caps[col_idx_rw, cap2_col]
        growth = np.where(is_last, ln2r, ln2p) \
            - np.where(_G_OF_C >= 1, ln1, 0.0) + mu_corr
        logz_sum = float(np.sum(growth, dtype=np.float64))
        gold_sum = float(np.sum(r["goldp"], dtype=np.float64))
        total += logz_sum - gold_sum
    return np.float32(total)


# revision 34
# speedup vs baseline: 1.0164x; 1.0164x over previous
"""BiLSTM-CRF negative log-likelihood kernel for 8 Trainium2 NeuronCores.

Strategy (data parallel over batch, 64 sequences per core):

logZ via PARALLEL SEGMENTS: the CRF forward chain contracts in direction
~10x per step (Birkhoff), so each sequence's 2048-step chain is split into
G=51 segments (seg 0: steps [0,48) started exactly from e_START; segs g>=1:
window [40g+8, 40g+48) preceded by K=8 burn-in steps from a uniform vector).
All 51*64 = 3264 segment-chains per core run simultaneously as columns of
[128, 816] tiles (4 groups of 32 tags stacked on partitions), so the serial
depth is 48 matmul+mul steps instead of 2048. Per-column log-growth between
two "captures" (V^T @ state matmuls at idx 8 and 48, with a plain-sum row
and an exp(trans[STOP]) -weighted row per group) telescopes into logZ.
No renorm is needed over 48 steps (bf16/fp32 exponent range suffices);
host adds back the constant MU per step and sums.

Gold score: emission = sum(feats * onehot(tags)) on device (multiply on
gpsimd, per-partition reduce on the scalar engine via activation accum;
last chunk as fused vector stt pieces in the chain-mul wait gaps);
transitions = <trans, C> on device where C is the [32,32] tag-pair count
histogram (integer preprocessing of tags, marshalled host-side like the
one-hot masks); stop term via a masked reduce of trans[STOP,:]. Device
outputs ln-captures and gold partials; host sums.

All inputs ride 3 DMAs: one combined per-chunk stream (raw feats slots +
window-only one-hot slots interleaved per chunk) and two packed param
tensors - dma_start issue time on the Sync queue is ~600 ns each, so
fewer, bigger DMAs matter.
"""

import sys

sys.path.insert(0, "/opt/trn_rl_repo")

import numpy as np
import ml_dtypes

B, S, T = 512, 2048, 32
START_IDX, STOP_IDX = 30, 31
N_CORES = 8
BC = B // N_CORES           # 64 sequences per core
K_BURN = 8
L_WIN = 40
DEPTH = K_BURN + L_WIN      # 48 serial steps
G_SEG = 51                  # 2048 = DEPTH + (G_SEG-1)*L_WIN
C_TOT = G_SEG * BC          # 3264 columns
NGRP = 4                    # tag-groups stacked on partitions
CPG = C_TOT // NGRP         # 816 columns per partition-group row
NSTR = 2                    # independent chain streams
CPS = CPG // NSTR           # 408 columns per stream
IC = 8                      # idx per streamed chunk
NCH = DEPTH // IC           # 6 chunks
HC = IC // 2
MU = float(np.log(32.0) + 1.0)
NEG = -10000.0
CCORR = BC * K_BURN // NGRP  # 128 corr-cell columns
# combined stream slot layout: chunk 0 -> 8 raw slots; chunks 1..5 ->
# 8 raw slots + 8 mc slots each
COMB_SLOTS = IC + (NCH - 1) * 2 * IC   # 88
# packed fp32 params: transT4 | tstop | cmat | trans
P32_W = 128 + 1 + T + T
# packed bf16 params: init | maskstop | rawc | mcc
PBF_W = CPG + BC + CCORR + CCORR

BF16 = ml_dtypes.bfloat16


def _build_program():
    import concourse.bass as bass
    import concourse.tile as tile
    from concourse import bacc, mybir

    dt = mybir.dt
    AF = mybir.ActivationFunctionType
    ALU = mybir.AluOpType
    AX = mybir.AxisListType

    nc = bacc.Bacc("TRN2", target_bir_lowering=False, debug=False,
                   num_devices=N_CORES)

    rawmc_d = nc.dram_tensor("rawmc", [128, COMB_SLOTS, CPG], dt.bfloat16,
                             kind="ExternalInput").ap()
    p32_d = nc.dram_tensor("p32", [128, P32_W], dt.float32,
                           kind="ExternalInput").ap()
    pbf_d = nc.dram_tensor("pbf", [128, PBF_W], dt.bfloat16,
                           kind="ExternalInput").ap()

    caps_d = nc.dram_tensor("caps", [8, 2 * CPG], dt.float32,
                            kind="ExternalOutput").ap()
    goldp_d = nc.dram_tensor("goldp", [128, 1], dt.float32,
                             kind="ExternalOutput").ap()

    with tile.TileContext(nc) as tc:
        with (
            tc.tile_pool(name="singles", bufs=1) as singles,
            tc.tile_pool(name="stateA", bufs=2) as stateA,
            tc.tile_pool(name="stateB", bufs=2) as stateB,
            tc.tile_pool(name="combp", bufs=3) as combp,
            tc.tile_pool(name="ftp", bufs=3) as ftp,
            tc.tile_pool(name="scrp", bufs=4) as scrp,
            tc.tile_pool(name="scrp2", bufs=1) as scrp2,
            tc.tile_pool(name="tailp", bufs=1) as tailp,
            tc.tile_pool(name="psA", bufs=1, space="PSUM") as psA,
            tc.tile_pool(name="psB", bufs=1, space="PSUM") as psB,
            tc.tile_pool(name="psc", bufs=2, space="PSUM") as psc,
        ):
            # ---------- chunk-0 stream DMA first, then params ----------
            # first half-chunk separately so ftile slice A starts ASAP
            comb0 = combp.tile([128, IC, CPG], dt.bfloat16, tag="c0")
            nc.sync.dma_start(comb0[:, 0:HC, :], rawmc_d[:, 0:HC, :])
            pbf = singles.tile([128, PBF_W], dt.bfloat16)
            nc.sync.dma_start(pbf[:, :], pbf_d[:, :])
            p32 = singles.tile([128, P32_W], dt.float32)
            nc.sync.dma_start(p32[:, :], p32_d[:, :])
            nc.sync.dma_start(comb0[:, HC:IC, :], rawmc_d[:, HC:IC, :])

            maskstop = pbf[0:T, CPG:CPG + BC]
            rawc = pbf[:, CPG + BC:CPG + BC + CCORR]
            mcc = pbf[:, CPG + BC + CCORR:PBF_W]
            tstop = p32[0:T, 128:129]
            cmat = p32[0:T, 129:129 + T]
            trans32 = p32[0:T, 129 + T:P32_W]

            # chain stationary: blockdiag4(exp(transT)) in bf16
            E4 = singles.tile([128, 128], dt.bfloat16)
            nc.scalar.activation(E4[:, :], p32[:, 0:128], AF.Exp)

            mub = singles.tile([128, 1], dt.float32)
            nc.vector.memset(mub[:, :], -MU)

            # capture stationary V [128, 8]: col 2q = ones on group q,
            # col 2q+1 = exp(trans[STOP,:]) on group q
            V = singles.tile([128, 8], dt.bfloat16)
            nc.vector.memset(V[:, :], 0.0)
            for q in range(NGRP):
                nc.vector.memset(V[32 * q:32 * q + 32, 2 * q:2 * q + 1], 1.0)
                nc.scalar.activation(V[32 * q:32 * q + 32, 2 * q + 1:2 * q + 2],
                                     tstop, AF.Exp)

            # gold accumulator slots
            NSLOT = 24
            eaccs = singles.tile([128, NSLOT], dt.float32)
            nc.vector.memset(eaccs[:, :], 0.0)
            slot = [0]

            def next_slot():
                s = slot[0]
                slot[0] += 1
                assert s < NSLOT
                return s

            # initial states
            stA = stateA.tile([128, CPS], dt.bfloat16, tag="stA")
            nc.vector.tensor_copy(stA[:, :], pbf[:, 0:CPS])
            stB = stateB.tile([128, CPS], dt.bfloat16, tag="stB")
            nc.vector.tensor_copy(stB[:, :], pbf[:, CPS:CPG])

            # captures land in SBUF immediately (PSUM bank budget)
            capsb = singles.tile([8, 2 * CPG], dt.float32)

            def do_capture(cidx, sA, sB):
                for s, st_s in ((0, sA), (1, sB)):
                    pc = psc.tile([8, CPS], dt.float32, tag="cap")
                    nc.tensor.matmul(pc[:, :], V[:, :], st_s[:, :],
                                     start=True, stop=True)
                    dst = capsb[:, cidx * CPG + s * CPS:
                                cidx * CPG + (s + 1) * CPS]
                    nc.vector.tensor_copy(dst, pc[:, :])

            pending_reduce = []

            # ---------- main loop ----------
            for ck in range(NCH):
                i0 = ck * IC
                if ck == 0:
                    comb = comb0
                else:
                    comb = combp.tile([128, 2 * IC, CPG], dt.bfloat16,
                                      tag="comb")
                    s0 = IC + (ck - 1) * 2 * IC
                    nc.sync.dma_start(comb[:, :, :],
                                      rawmc_d[:, s0:s0 + 2 * IC, :])
                raw = comb[:, 0:IC, :]
                mc = None if ck == 0 else comb[:, IC:2 * IC, :]

                # emission multiplies on gpsimd; reduces go to the scalar
                # engine one chunk later. Last chunk: fused vector stt
                # pieces interleaved into the chain-mul wait gaps.
                if 1 <= ck < NCH - 1:
                    for (a, b) in ((0, HC), (HC, IC)):
                        scr = scrp.tile([128, HC, CPG], dt.bfloat16,
                                        tag="scr")
                        nc.gpsimd.tensor_mul(scr[:, :, :],
                                             comb[:, IC + a:IC + b, :],
                                             comb[:, a:b, :])
                        pending_reduce.append(scr[:, :, :])

                # chain: 8 steps, 2 streams; ftile in half-chunk slices
                for j in range(IC):
                    i = i0 + j
                    if j % HC == 0:
                        ftile = ftp.tile([128, HC, CPG], dt.bfloat16,
                                         tag="ft")
                        nc.scalar.activation(ftile[:, :, :],
                                             comb[:, j:j + HC, :], AF.Exp,
                                             bias=mub[:, :])
                    if i == K_BURN:
                        do_capture(0, stA, stB)
                    puA = psA.tile([128, CPS], dt.float32, tag="puA")
                    nc.tensor.matmul(puA[:, :], E4[:, :], stA[:, :],
                                     start=True, stop=True)
                    puB = psB.tile([128, CPS], dt.float32, tag="puB")
                    nc.tensor.matmul(puB[:, :], E4[:, :], stB[:, :],
                                     start=True, stop=True)
                    stA = stateA.tile([128, CPS], dt.bfloat16, tag="stA")
                    nc.vector.tensor_mul(stA[:, :], puA[:, :],
                                         ftile[:, j % HC, 0:CPS])
                    stB = stateB.tile([128, CPS], dt.bfloat16, tag="stB")
                    nc.vector.tensor_mul(stB[:, :], puB[:, :],
                                         ftile[:, j % HC, CPS:CPG])
                    if ck == NCH - 1:
                        scr = scrp2.tile([128, CPG], dt.bfloat16, tag="scrl")
                        nc.vector.scalar_tensor_tensor(
                            scr[:, :], raw[:, j, :], 1.0, mc[:, j, :],
                            op0=ALU.mult, op1=ALU.mult,
                            accum_out=eaccs[:, next_slot():slot[0]])
                    if j == HC and pending_reduce:
                        for scr_ap in pending_reduce:
                            scr2 = scrp2.tile([128, HC, CPG], dt.bfloat16,
                                              tag="sc2")
                            nc.scalar.activation(
                                scr2[:, :, :], scr_ap, AF.Identity,
                                accum_out=eaccs[:, next_slot():slot[0]])
                        pending_reduce.clear()

            for scr_ap in pending_reduce:
                scr2 = scrp2.tile([128, HC, CPG], dt.bfloat16, tag="sc2")
                nc.scalar.activation(scr2[:, :, :], scr_ap, AF.Identity,
                                     accum_out=eaccs[:, next_slot():slot[0]])
            pending_reduce.clear()

            # capture 2: final states
            do_capture(1, stA, stB)

            # ---------- gold tails ----------
            scr2 = tailp.tile([128, CCORR], dt.bfloat16)
            nc.vector.scalar_tensor_tensor(
                scr2[:, :], rawc, 1.0, mcc,
                op0=ALU.mult, op1=ALU.mult,
                accum_out=eaccs[:, next_slot():slot[0]])
            scr3 = tailp.tile([T, T], dt.float32)
            nc.vector.scalar_tensor_tensor(
                scr3[:, :], trans32, 1.0, cmat,
                op0=ALU.mult, op1=ALU.mult,
                accum_out=eaccs[0:T, next_slot():slot[0]])
            scr4 = tailp.tile([T, BC], dt.float32)
            nc.vector.scalar_tensor_tensor(
                scr4[:, :], maskstop, tstop, maskstop,
                op0=ALU.mult, op1=ALU.mult,
                accum_out=eaccs[0:T, next_slot():slot[0]])

            gp = tailp.tile([128, 1], dt.float32)
            nc.vector.tensor_reduce(gp[:, :], eaccs[:, :], axis=AX.X,
                                    op=ALU.add)
            nc.sync.dma_start(goldp_d[:, :], gp[:, :])

            # ---------- captures: ln + out ----------
            capln = tailp.tile([8, 2 * CPG], dt.float32)
            nc.scalar.activation(capln[:, :], capsb[:, :], AF.Ln)
            nc.sync.dma_start(caps_d[:, :], capln[:, :])

    nc.compile()
    return nc


def _marshal(feats, transitions, tags):
    feats = np.asarray(feats, dtype=np.float32)
    trans = np.asarray(transitions, dtype=np.float32)
    tags = np.asarray(tags)
    eye = np.arange(T, dtype=tags.dtype)

    g_of_c = np.arange(C_TOT) // BC
    b_of_c = np.arange(C_TOT) % BC
    # cell time: t(i, g) = L_WIN*g + i
    t_cell = (L_WIN * g_of_c)[:, None] + np.arange(DEPTH)[None, :]  # [C,48]

    p32 = np.full((128, P32_W), NEG, dtype=np.float32)
    tq = np.ascontiguousarray(trans.T)
    for q in range(NGRP):
        p32[32 * q:32 * q + 32, 32 * q:32 * q + 32] = tq
    p32[:, 128:] = 0.0
    p32[0:T, 128] = trans[STOP_IDX, :]
    p32[0:T, 129 + T:P32_W] = trans

    in_maps = []
    for c in range(N_CORES):
        b0, b1 = c * BC, (c + 1) * BC
        f = feats[b0:b1]          # [64, 2048, 32]
        tg = tags[b0:b1]          # [64, 2048]

        cells = f[b_of_c[:, None], t_cell, :]            # [C, 48, 32]
        raw = cells.reshape(NGRP, CPG, DEPTH, T) \
            .transpose(0, 3, 2, 1).reshape(128, DEPTH, CPG).astype(BF16)

        tw = tg[b_of_c[:, None], t_cell[:, K_BURN:]]     # [C, 40] window
        mcw = (tw[:, :, None] == eye).astype(BF16)        # [C, 40, 32]
        mcw = mcw.reshape(NGRP, CPG, DEPTH - K_BURN, T) \
            .transpose(0, 3, 2, 1).reshape(128, DEPTH - K_BURN, CPG)

        rawmc = np.empty((128, COMB_SLOTS, CPG), dtype=BF16)
        rawmc[:, 0:IC] = raw[:, 0:IC]
        for ck in range(1, NCH):
            s0 = IC + (ck - 1) * 2 * IC
            rawmc[:, s0:s0 + IC] = raw[:, ck * IC:(ck + 1) * IC]
            rawmc[:, s0 + IC:s0 + 2 * IC] = \
                mcw[:, (ck - 1) * IC:ck * IC]

        init_cols = np.full((C_TOT, T), 1.0 / T, dtype=np.float32)
        init_cols[g_of_c == 0] = 0.0
        init_cols[g_of_c == 0, START_IDX] = 1.0
        init = init_cols.reshape(NGRP, CPG, T).transpose(0, 2, 1) \
            .reshape(128, CPG)

        bb = np.arange(BC * K_BURN) // K_BURN
        tt = np.arange(BC * K_BURN) % K_BURN
        rawc = f[bb, tt, :].reshape(NGRP, CCORR, T).transpose(0, 2, 1) \
            .reshape(128, CCORR)
        mcc = (tg[bb, tt][:, None] == eye) \
            .reshape(NGRP, CCORR, T).transpose(0, 2, 1).reshape(128, CCORR)

        pbf = np.zeros((128, PBF_W), dtype=BF16)
        pbf[:, 0:CPG] = init.astype(BF16)
        pbf[0:T, CPG:CPG + BC] = (tg[:, S - 1, None] == eye).T.astype(BF16)
        pbf[:, CPG + BC:CPG + BC + CCORR] = rawc.astype(BF16)
        pbf[:, CPG + BC + CCORR:PBF_W] = mcc.astype(BF16)

        # pair-count histogram C[i, j] = #{t: tag_t = i, tag_{t-1} = j}
        prev = np.concatenate(
            [np.full((BC, 1), START_IDX, dtype=tg.dtype), tg[:, :-1]], axis=1)
        pair = (tg.astype(np.int64) * T + prev.astype(np.int64)).ravel()
        cmat = np.bincount(pair, minlength=T * T).reshape(T, T)
        p32c = p32.copy()
        p32c[0:T, 129:129 + T] = cmat.astype(np.float32)

        in_maps.append({"rawmc": rawmc, "p32": p32c, "pbf": pbf})
    return in_maps


_PROGRAM = [None]
TRACE = False
TRACE_KW = {}
LAST_EXEC_NS = None
LAST_RESULT = [None]

_G_OF_C = np.arange(C_TOT) // BC
_GRP_OF_C = np.arange(C_TOT) // CPG
_J_OF_C = np.arange(C_TOT) % CPG
_S_OF_C = _J_OF_C // CPS
_JJ_OF_C = _J_OF_C % CPS


def kernel(feats, transitions, tags):
    global LAST_EXEC_NS
    from concourse.bass_utils import run_bass_kernel_spmd

    if _PROGRAM[0] is None:
        _PROGRAM[0] = _build_program()
    nc = _PROGRAM[0]
    in_maps = _marshal(feats, transitions, tags)
    res = run_bass_kernel_spmd(nc, in_maps, list(range(N_CORES)),
                               trace=TRACE, **TRACE_KW)
    LAST_EXEC_NS = res.exec_time_ns
    LAST_RESULT[0] = res

    col_idx = _GRP_OF_C * 2
    col_idx_rw = _GRP_OF_C * 2 + 1
    cap1_col = 0 * CPG + _S_OF_C * CPS + _JJ_OF_C
    cap2_col = 1 * CPG + _S_OF_C * CPS + _JJ_OF_C
    is_last = _G_OF_C == G_SEG - 1
    mu_corr = np.where(_G_OF_C == 0, DEPTH * MU, L_WIN * MU)

    total = 0.0
    for c in range(N_CORES):
        r = res.results[c]
        caps = r["caps"]            # [8, 2*CPG]
        ln1 = caps[col_idx, cap1_col]
        ln2p = caps[col_idx, cap2_col]
        ln2r = caps[col_idx_rw, cap2_col]
        growth = np.where(is_last, ln2r, ln2p) \
            - np.where(_G_OF_C >= 1, ln1, 0.0) + mu_corr
        logz_sum = float(np.sum(growth, dtype=np.float64))
        gold_sum = float(np.sum(r["goldp"], dtype=np.float64))
        total += logz_sum - gold_sum
    return np.float32(total)


# revision 35
# speedup vs baseline: 1.1437x; 1.1253x over previous
"""BiLSTM-CRF negative log-likelihood kernel for 8 Trainium2 NeuronCores.

Strategy (data parallel over batch, 64 sequences per core):

logZ via PARALLEL SEGMENTS: the CRF forward chain contracts in direction
~10x per step (Birkhoff), so each sequence's 2048-step chain is split into
G=51 segments (seg 0: steps [0,48) started exactly from e_START; segs g>=1:
window [40g+8, 40g+48) preceded by K=8 burn-in steps from a uniform vector).
All 51*64 = 3264 segment-chains per core run simultaneously as columns of
[128, 816] tiles (4 groups of 32 tags stacked on partitions), so the serial
depth is 48 matmul+mul steps instead of 2048. Per-column log-growth between
two "captures" (V^T @ state matmuls at idx 8 and 48, with a plain-sum row
and an exp(trans[STOP]) -weighted row per group) telescopes into logZ.
No renorm is needed over 48 steps (bf16/fp32 exponent range suffices);
host adds back the constant MU per step and sums.

Gold score: emission = sum(feats * onehot(tags)) on device (multiply on
gpsimd, per-partition reduce on the scalar engine via activation accum;
last chunk as fused vector stt pieces in the chain-mul wait gaps);
transitions = <trans, C> on device where C is the [32,32] tag-pair count
histogram (integer preprocessing of tags, marshalled host-side like the
one-hot masks); stop term via a masked reduce of trans[STOP,:]. Device
outputs ln-captures and gold partials; host sums.

All inputs ride 3 DMAs: one combined per-chunk stream (raw feats slots +
window-only one-hot slots interleaved per chunk) and two packed param
tensors - dma_start issue time on the Sync queue is ~600 ns each, so
fewer, bigger DMAs matter.
"""

import sys

sys.path.insert(0, "/opt/trn_rl_repo")

import numpy as np
import ml_dtypes

B, S, T = 512, 2048, 32
START_IDX, STOP_IDX = 30, 31
N_CORES = 8
BC = B // N_CORES           # 64 sequences per core
K_BURN = 8
L_WIN = 40
DEPTH = K_BURN + L_WIN      # 48 serial steps
G_SEG = 51                  # 2048 = DEPTH + (G_SEG-1)*L_WIN
C_TOT = G_SEG * BC          # 3264 columns
NGRP = 4                    # tag-groups stacked on partitions
CPG = C_TOT // NGRP         # 816 columns per partition-group row
NSTR = 2                    # independent chain streams
CPS = CPG // NSTR           # 408 columns per stream
IC = 8                      # idx per streamed chunk
NCH = DEPTH // IC           # 6 chunks
HC = IC // 2
MU = float(np.log(32.0) + 1.0)
NEG = -10000.0
CCORR = BC * K_BURN // NGRP  # 128 corr-cell columns
# combined stream slot layout: chunk 0 -> 8 raw slots; chunks 1..5 ->
# 8 raw slots + 8 mc slots each
COMB_SLOTS = IC + (NCH - 1) * 2 * IC   # 88
# packed fp32 params: transT4 | tstop | cmat | trans
P32_W = 128 + 1 + T + T
# packed bf16 params: init | maskstop | rawc | mcc
PBF_W = CPG + BC + CCORR + CCORR

BF16 = ml_dtypes.bfloat16


def _build_program():
    import concourse.bass as bass
    import concourse.tile as tile
    from concourse import bacc, mybir

    dt = mybir.dt
    AF = mybir.ActivationFunctionType
    ALU = mybir.AluOpType
    AX = mybir.AxisListType

    nc = bacc.Bacc("TRN2", target_bir_lowering=False, debug=False,
                   num_devices=N_CORES)

    rawmc_d = nc.dram_tensor("rawmc", [128, COMB_SLOTS, CPG], dt.bfloat16,
                             kind="ExternalInput").ap()
    p32_d = nc.dram_tensor("p32", [128, P32_W], dt.float32,
                           kind="ExternalInput").ap()
    pbf_d = nc.dram_tensor("pbf", [128, PBF_W], dt.bfloat16,
                           kind="ExternalInput").ap()

    caps_d = nc.dram_tensor("caps", [8, 2 * CPG], dt.float32,
                            kind="ExternalOutput").ap()
    goldp_d = nc.dram_tensor("goldp", [128, 1], dt.float32,
                             kind="ExternalOutput").ap()

    with tile.TileContext(nc) as tc:
        with (
            tc.tile_pool(name="singles", bufs=1) as singles,
            tc.tile_pool(name="stateA", bufs=2) as stateA,
            tc.tile_pool(name="stateB", bufs=2) as stateB,
            tc.tile_pool(name="combp", bufs=3) as combp,
            tc.tile_pool(name="ftp", bufs=3) as ftp,
            tc.tile_pool(name="scrp", bufs=4) as scrp,
            tc.tile_pool(name="scrp2", bufs=1) as scrp2,
            tc.tile_pool(name="tailp", bufs=1) as tailp,
            tc.tile_pool(name="psA", bufs=1, space="PSUM") as psA,
            tc.tile_pool(name="psB", bufs=1, space="PSUM") as psB,
            tc.tile_pool(name="psc", bufs=2, space="PSUM") as psc,
        ):
            # ---------- chunk-0 stream DMA first, then params ----------
            comb0 = combp.tile([128, IC, CPG], dt.bfloat16, tag="c0")
            nc.sync.dma_start(comb0[:, :, :], rawmc_d[:, 0:IC, :])
            pbf = singles.tile([128, PBF_W], dt.bfloat16)
            nc.sync.dma_start(pbf[:, :], pbf_d[:, :])
            p32 = singles.tile([128, P32_W], dt.float32)
            nc.sync.dma_start(p32[:, :], p32_d[:, :])

            maskstop = pbf[0:T, CPG:CPG + BC]
            rawc = pbf[:, CPG + BC:CPG + BC + CCORR]
            mcc = pbf[:, CPG + BC + CCORR:PBF_W]
            tstop = p32[0:T, 128:129]
            cmat = p32[0:T, 129:129 + T]
            trans32 = p32[0:T, 129 + T:P32_W]

            # chain stationary: blockdiag4(exp(transT)) in bf16
            E4 = singles.tile([128, 128], dt.bfloat16)
            nc.scalar.activation(E4[:, :], p32[:, 0:128], AF.Exp)

            mub = singles.tile([128, 1], dt.float32)
            nc.vector.memset(mub[:, :], -MU)

            # capture stationary V [128, 8]: col 2q = ones on group q,
            # col 2q+1 = exp(trans[STOP,:]) on group q
            V = singles.tile([128, 8], dt.bfloat16)
            nc.vector.memset(V[:, :], 0.0)
            for q in range(NGRP):
                nc.vector.memset(V[32 * q:32 * q + 32, 2 * q:2 * q + 1], 1.0)
                nc.scalar.activation(V[32 * q:32 * q + 32, 2 * q + 1:2 * q + 2],
                                     tstop, AF.Exp)

            # gold accumulator slots
            NSLOT = 24
            eaccs = singles.tile([128, NSLOT], dt.float32)
            nc.vector.memset(eaccs[:, :], 0.0)
            slot = [0]

            def next_slot():
                s = slot[0]
                slot[0] += 1
                assert s < NSLOT
                return s

            # initial states
            stA = stateA.tile([128, CPS], dt.bfloat16, tag="stA")
            nc.vector.tensor_copy(stA[:, :], pbf[:, 0:CPS])
            stB = stateB.tile([128, CPS], dt.bfloat16, tag="stB")
            nc.vector.tensor_copy(stB[:, :], pbf[:, CPS:CPG])

            # captures land in SBUF immediately (PSUM bank budget)
            capsb = singles.tile([8, 2 * CPG], dt.float32)

            def do_capture(cidx, sA, sB):
                for s, st_s in ((0, sA), (1, sB)):
                    pc = psc.tile([8, CPS], dt.float32, tag="cap")
                    nc.tensor.matmul(pc[:, :], V[:, :], st_s[:, :],
                                     start=True, stop=True)
                    dst = capsb[:, cidx * CPG + s * CPS:
                                cidx * CPG + (s + 1) * CPS]
                    nc.vector.tensor_copy(dst, pc[:, :])

            pending_reduce = []

            # ---------- main loop ----------
            for ck in range(NCH):
                i0 = ck * IC
                if ck == 0:
                    comb = comb0
                else:
                    comb = combp.tile([128, 2 * IC, CPG], dt.bfloat16,
                                      tag="comb")
                    s0 = IC + (ck - 1) * 2 * IC
                    nc.sync.dma_start(comb[:, :, :],
                                      rawmc_d[:, s0:s0 + 2 * IC, :])
                raw = comb[:, 0:IC, :]
                mc = None if ck == 0 else comb[:, IC:2 * IC, :]

                # emission multiplies on gpsimd; reduces go to the scalar
                # engine one chunk later. Last chunk: fused vector stt
                # pieces interleaved into the chain-mul wait gaps.
                if 1 <= ck < NCH - 1:
                    for (a, b) in ((0, HC), (HC, IC)):
                        scr = scrp.tile([128, HC, CPG], dt.bfloat16,
                                        tag="scr")
                        nc.gpsimd.tensor_mul(scr[:, :, :],
                                             comb[:, IC + a:IC + b, :],
                                             comb[:, a:b, :])
                        pending_reduce.append(scr[:, :, :])

                # chain: 8 steps, 2 streams; ftile in half-chunk slices
                for j in range(IC):
                    i = i0 + j
                    if j % HC == 0:
                        ftile = ftp.tile([128, HC, CPG], dt.bfloat16,
                                         tag="ft")
                        nc.scalar.activation(ftile[:, :, :],
                                             comb[:, j:j + HC, :], AF.Exp,
                                             bias=mub[:, :])
                    if i == K_BURN:
                        do_capture(0, stA, stB)
                    puA = psA.tile([128, CPS], dt.float32, tag="puA")
                    nc.tensor.matmul(puA[:, :], E4[:, :], stA[:, :],
                                     start=True, stop=True)
                    puB = psB.tile([128, CPS], dt.float32, tag="puB")
                    nc.tensor.matmul(puB[:, :], E4[:, :], stB[:, :],
                                     start=True, stop=True)
                    stA = stateA.tile([128, CPS], dt.bfloat16, tag="stA")
                    nc.vector.tensor_mul(stA[:, :], puA[:, :],
                                         ftile[:, j % HC, 0:CPS])
                    stB = stateB.tile([128, CPS], dt.bfloat16, tag="stB")
                    nc.vector.tensor_mul(stB[:, :], puB[:, :],
                                         ftile[:, j % HC, CPS:CPG])
                    if ck == NCH - 1:
                        scr = scrp2.tile([128, CPG], dt.bfloat16, tag="scrl")
                        nc.vector.scalar_tensor_tensor(
                            scr[:, :], raw[:, j, :], 1.0, mc[:, j, :],
                            op0=ALU.mult, op1=ALU.mult,
                            accum_out=eaccs[:, next_slot():slot[0]])
                    if j == HC and pending_reduce:
                        for scr_ap in pending_reduce:
                            scr2 = scrp2.tile([128, HC, CPG], dt.bfloat16,
                                              tag="sc2")
                            nc.scalar.activation(
                                scr2[:, :, :], scr_ap, AF.Identity,
                                accum_out=eaccs[:, next_slot():slot[0]])
                        pending_reduce.clear()

            for scr_ap in pending_reduce:
                scr2 = scrp2.tile([128, HC, CPG], dt.bfloat16, tag="sc2")
                nc.scalar.activation(scr2[:, :, :], scr_ap, AF.Identity,
                                     accum_out=eaccs[:, next_slot():slot[0]])
            pending_reduce.clear()

            # capture 2: final states
            do_capture(1, stA, stB)

            # ---------- gold tails ----------
            scr2 = tailp.tile([128, CCORR], dt.bfloat16)
            nc.vector.scalar_tensor_tensor(
                scr2[:, :], rawc, 1.0, mcc,
                op0=ALU.mult, op1=ALU.mult,
                accum_out=eaccs[:, next_slot():slot[0]])
            scr3 = tailp.tile([T, T], dt.float32)
            nc.vector.scalar_tensor_tensor(
                scr3[:, :], trans32, 1.0, cmat,
                op0=ALU.mult, op1=ALU.mult,
                accum_out=eaccs[0:T, next_slot():slot[0]])
            scr4 = tailp.tile([T, BC], dt.float32)
            nc.vector.scalar_tensor_tensor(
                scr4[:, :], maskstop, tstop, maskstop,
                op0=ALU.mult, op1=ALU.mult,
                accum_out=eaccs[0:T, next_slot():slot[0]])

            gp = tailp.tile([128, 1], dt.float32)
            nc.vector.tensor_reduce(gp[:, :], eaccs[:, :], axis=AX.X,
                                    op=ALU.add)
            nc.sync.dma_start(goldp_d[:, :], gp[:, :])

            # ---------- captures: ln + out ----------
            capln = tailp.tile([8, 2 * CPG], dt.float32)
            nc.scalar.activation(capln[:, :], capsb[:, :], AF.Ln)
            nc.sync.dma_start(caps_d[:, :], capln[:, :])

    nc.compile()
    return nc


def _marshal(feats, transitions, tags):
    feats = np.asarray(feats, dtype=np.float32)
    trans = np.asarray(transitions, dtype=np.float32)
    tags = np.asarray(tags)
    eye = np.arange(T, dtype=tags.dtype)

    g_of_c = np.arange(C_TOT) // BC
    b_of_c = np.arange(C_TOT) % BC
    # cell time: t(i, g) = L_WIN*g + i
    t_cell = (L_WIN * g_of_c)[:, None] + np.arange(DEPTH)[None, :]  # [C,48]

    p32 = np.full((128, P32_W), NEG, dtype=np.float32)
    tq = np.ascontiguousarray(trans.T)
    for q in range(NGRP):
        p32[32 * q:32 * q + 32, 32 * q:32 * q + 32] = tq
    p32[:, 128:] = 0.0
    p32[0:T, 128] = trans[STOP_IDX, :]
    p32[0:T, 129 + T:P32_W] = trans

    in_maps = []
    for c in range(N_CORES):
        b0, b1 = c * BC, (c + 1) * BC
        f = feats[b0:b1]          # [64, 2048, 32]
        tg = tags[b0:b1]          # [64, 2048]

        cells = f[b_of_c[:, None], t_cell, :]            # [C, 48, 32]
        raw = cells.reshape(NGRP, CPG, DEPTH, T) \
            .transpose(0, 3, 2, 1).reshape(128, DEPTH, CPG).astype(BF16)

        tw = tg[b_of_c[:, None], t_cell[:, K_BURN:]]     # [C, 40] window
        mcw = (tw[:, :, None] == eye).astype(BF16)        # [C, 40, 32]
        mcw = mcw.reshape(NGRP, CPG, DEPTH - K_BURN, T) \
            .transpose(0, 3, 2, 1).reshape(128, DEPTH - K_BURN, CPG)

        rawmc = np.empty((128, COMB_SLOTS, CPG), dtype=BF16)
        rawmc[:, 0:IC] = raw[:, 0:IC]
        for ck in range(1, NCH):
            s0 = IC + (ck - 1) * 2 * IC
            rawmc[:, s0:s0 + IC] = raw[:, ck * IC:(ck + 1) * IC]
            rawmc[:, s0 + IC:s0 + 2 * IC] = \
                mcw[:, (ck - 1) * IC:ck * IC]

        init_cols = np.full((C_TOT, T), 1.0 / T, dtype=np.float32)
        init_cols[g_of_c == 0] = 0.0
        init_cols[g_of_c == 0, START_IDX] = 1.0
        init = init_cols.reshape(NGRP, CPG, T).transpose(0, 2, 1) \
            .reshape(128, CPG)

        bb = np.arange(BC * K_BURN) // K_BURN
        tt = np.arange(BC * K_BURN) % K_BURN
        rawc = f[bb, tt, :].reshape(NGRP, CCORR, T).transpose(0, 2, 1) \
            .reshape(128, CCORR)
        mcc = (tg[bb, tt][:, None] == eye) \
            .reshape(NGRP, CCORR, T).transpose(0, 2, 1).reshape(128, CCORR)

        pbf = np.zeros((128, PBF_W), dtype=BF16)
        pbf[:, 0:CPG] = init.astype(BF16)
        pbf[0:T, CPG:CPG + BC] = (tg[:, S - 1, None] == eye).T.astype(BF16)
        pbf[:, CPG + BC:CPG + BC + CCORR] = rawc.astype(BF16)
        pbf[:, CPG + BC + CCORR:PBF_W] = mcc.astype(BF16)

        # pair-count histogram C[i, j] = #{t: tag_t = i, tag_{t-1} = j}
        prev = np.concatenate(
            [np.full((BC, 1), START_IDX, dtype=tg.dtype), tg[:, :-1]], axis=1)
        pair = (tg.astype(np.int64) * T + prev.astype(np.int64)).ravel()
        cmat = np.bincount(pair, minlength=T * T).reshape(T, T)
        p32c = p32.copy()
        p32c[0:T, 129:129 + T] = cmat.astype(np.float32)

        in_maps.append({"rawmc": rawmc, "p32": p32c, "pbf": pbf})
    return in_maps


_PROGRAM = [None]
TRACE = False
TRACE_KW = {}
LAST_EXEC_NS = None
LAST_RESULT = [None]

_G_OF_C = np.arange(C_TOT) // BC
_GRP_OF_C = np.arange(C_TOT) // CPG
_J_OF_C = np.arange(C_TOT) % CPG
_S_OF_C = _J_OF_C // CPS
_JJ_OF_C = _J_OF_C % CPS


def kernel(feats, transitions, tags):
    global LAST_EXEC_NS
    from concourse.bass_utils import run_bass_kernel_spmd

    if _PROGRAM[0] is None:
        _PROGRAM[0] = _build_program()
    nc = _PROGRAM[0]
    in_maps = _marshal(feats, transitions, tags)
    res = run_bass_kernel_spmd(nc, in_maps, list(range(N_CORES)),
                               trace=TRACE, **TRACE_KW)
    LAST_EXEC_NS = res.exec_time_ns
    LAST_RESULT[0] = res

    col_idx = _GRP_OF_C * 2
    col_idx_rw = _GRP_OF_C * 2 + 1
    cap1_col = 0 * CPG + _S_OF_C * CPS + _JJ_OF_C
    cap2_col = 1 * CPG + _S_OF_C * CPS + _JJ_OF_C
    is_last = _G_OF_C == G_SEG - 1
    mu_corr = np.where(_G_OF_C == 0, DEPTH * MU, L_WIN * MU)

    total = 0.0
    for c in range(N_CORES):
        r = res.results[c]
        caps = r["caps"]            # [8, 2*CPG]
        ln1 = caps[col_idx, cap1_col]
        ln2p = caps[col_idx, cap2_col]
        ln2r = caps[col_idx_rw, cap2_col]
        growth = np.where(is_last, ln2r, ln2p) \
            - np.where(_G_OF_C >= 1, ln1, 0.0) + mu_corr
        logz_sum = float(np.sum(growth, dtype=np.float64))
        gold_sum = float(np.sum(r["goldp"], dtype=np.float64))
        total += logz_sum - gold_sum
    return np.float32(total)
